# revision 10
# baseline (speedup 1.0000x reference)
"""MultiHeadSelfAttention1D on 8 trn2 NeuronCores.

Sharding: core c handles batch b=c//4 and query-range [(c%4)*1024, +1024),
all 8 heads. Each core redundantly computes K/V for its batch (cheaper than
collectives) and produces a complete [C, 1024] slice of the output — no
cross-core communication at all.

Per-core dataflow (all matmul operands in float32r = TF32, PE full rate):
  qT/kT via  W.T@x  (transposed projections, K=256 accumulated over 2 chunks)
  v in token-major layout via x.T@Wv, stored per-head as [v_h | ones] (33 cols)
  S^T[j,i] tiles via 4 row-packed K=32 matmuls (tile_position row groups)
  P = exp(SCALE*S^T) on ScalarE, one [128,2048] ACTIVATE per 4-head group
  out^T + rowsum via M=33 matmuls accumulating over j (ones column -> rowsum)
  normalize on VectorE (reciprocal + broadcast mul), project with Wproj.
"""
import sys

sys.path.insert(0, '/opt/trn_rl_repo')

import numpy as np

B, C, N = 2, 256, 4096
H, HD = 8, 32
NI = N // 4          # queries per core
SCALE = HD ** -0.5

_CACHE = {}


def _build_nc():
    import concourse.bass as bass
    import concourse.tile as tile
    from concourse import bacc, mybir

    F32 = mybir.dt.float32
    F32R = mybir.dt.float32r
    AF = mybir.ActivationFunctionType

    nc = bacc.Bacc("TRN2", target_bir_lowering=False, debug=False, num_devices=8)
    XB = nc.declare_dram_parameter("xb", [C, N], F32, isOutput=False)
    XQ = nc.declare_dram_parameter("xq", [C, NI], F32, isOutput=False)
    WQKV = nc.declare_dram_parameter("wqkv", [C, 3 * C], F32, isOutput=False)
    BQKV = nc.declare_dram_parameter("bqkv", [1, 3 * C], F32, isOutput=False)
    WPROJ = nc.declare_dram_parameter("wproj", [C, C], F32, isOutput=False)
    BPROJ = nc.declare_dram_parameter("bproj", [1, C], F32, isOutput=False)
    OUT = nc.declare_dram_parameter("out", [C, NI], F32, isOutput=True)

    NJC = N // 512       # 8 kT column chunks
    NVC = N // 128       # 32 v token chunks
    NIC = NI // 512      # 2 query chunks

    with tile.TileContext(nc) as tc:
        with (
            tc.tile_pool(name="persist", bufs=1) as pers,
            tc.tile_pool(name="ps_st", bufs=1, space="PSUM") as ps_st,
            tc.tile_pool(name="ps_acc", bufs=4, space="PSUM") as ps_acc,
        ):
            # ---- load fp32, round to f32r via DVE copies ----
            xb = [pers.tile([128, N], F32R, name=f"xb{i}") for i in range(2)]
            xq = [pers.tile([128, NI], F32R, name=f"xq{i}") for i in range(2)]
            wqkv = [pers.tile([128, 3 * C], F32R, name=f"wqkv{i}") for i in range(2)]
            wproj = [pers.tile([128, C], F32R, name=f"wproj{i}") for i in range(2)]
            bqkv = pers.tile([1, 3 * C], F32R)
            bproj = pers.tile([1, C], F32R)
            with tc.tile_pool(name="stage", bufs=1) as stage:
                for i in range(2):
                    for dst, src, w in ((xb[i], XB, N), (xq[i], XQ, NI)):
                        st_t = stage.tile([128, w], F32, tag="big", name="stbig")
                        nc.sync.dma_start(st_t[:], src[128 * i:128 * i + 128, :])
                        nc.vector.tensor_copy(dst[:], st_t[:])
                for i in range(2):
                    wq_s = stage.tile([128, 3 * C], F32, name=f"wqs{i}")
                    nc.sync.dma_start(wq_s[:], WQKV[128 * i:128 * i + 128, :])
                    nc.vector.tensor_copy(wqkv[i][:], wq_s[:])
                    wp_s = stage.tile([128, C], F32, name=f"wps{i}")
                    nc.sync.dma_start(wp_s[:], WPROJ[128 * i:128 * i + 128, :])
                    nc.vector.tensor_copy(wproj[i][:], wp_s[:])
                bq_s = stage.tile([1, 3 * C], F32)
                bp_s = stage.tile([1, C], F32)
                nc.sync.dma_start(bq_s[:], BQKV[:])
                nc.sync.dma_start(bp_s[:], BPROJ[:])
                nc.vector.tensor_copy(bqkv[:], bq_s[:])
                nc.vector.tensor_copy(bproj[:], bp_s[:])

            work_cm = tc.tile_pool(name="work", bufs=2)
            work = work_cm.__enter__()
            ones_f32 = pers.tile([128, 512], F32)
            nc.vector.memset(ones_f32[:], 1.0)
            ones512 = pers.tile([1, 512], F32R)
            ones128 = pers.tile([1, 128], F32R)
            nc.vector.tensor_copy(ones512[:], ones_f32[0:1, :])
            nc.vector.tensor_copy(ones128[:], ones_f32[0:1, 0:128])

            # ---- projections ----
            kT = [pers.tile([128, N], F32R, name=f"kT{g}") for g in range(2)]
            qT = [pers.tile([128, NI], F32R, name=f"qT{g}") for g in range(2)]
            # v: per token-chunk, per head: [v_h(32) | ones(32)] -> M=64 lhsT,
            # so the PV matmul emits out^T rows 0:32 and rowsum x32 rows 32:64.
            vb = pers.tile([128, NVC, H, 64], F32R)
            for jc in range(NVC):
                nc.vector.tensor_copy(
                    vb[:, jc, :, 32:64],
                    ones_f32[:, 0:256].rearrange("p (h d) -> p h d", h=H))

            for g in range(2):
                co = C + 128 * g  # k block columns
                for jc in range(NJC):
                    acc = ps_acc.tile([128, 512], F32, tag="acc", name="kacc")
                    for cc in range(2):
                        nc.tensor.matmul(
                            acc[:], wqkv[cc][:, co:co + 128],
                            xb[cc][:, 512 * jc:512 * jc + 512],
                            start=(cc == 0), stop=False)
                    nc.tensor.matmul(acc[:], bqkv[0:1, co:co + 128], ones512[:],
                                     start=False, stop=True)
                    nc.scalar.activation(kT[g][:, 512 * jc:512 * jc + 512], acc[:], AF.Copy)
                for ic in range(NIC):
                    acc = ps_acc.tile([128, 512], F32, tag="acc", name="qacc")
                    for cc in range(2):
                        nc.tensor.matmul(
                            acc[:], wqkv[cc][:, 128 * g:128 * g + 128],
                            xq[cc][:, 512 * ic:512 * ic + 512],
                            start=(cc == 0), stop=False)
                    nc.tensor.matmul(acc[:], bqkv[0:1, 128 * g:128 * g + 128], ones512[:],
                                     start=False, stop=True)
                    nc.scalar.activation(qT[g][:, 512 * ic:512 * ic + 512], acc[:], AF.Copy)

            for jc in range(NVC):
                acc = ps_acc.tile([128, C], F32, tag="acc", name="vacc")
                for cc in range(2):
                    nc.tensor.matmul(
                        acc[:], xb[cc][:, 128 * jc:128 * jc + 128],
                        wqkv[cc][:, 2 * C:3 * C],
                        start=(cc == 0), stop=False)
                nc.tensor.matmul(acc[:], ones128[:], bqkv[0:1, 2 * C:3 * C],
                                 start=False, stop=True)
                nc.scalar.activation(
                    vb[:, jc, :, 0:32],
                    acc[:].rearrange("p (h d) -> p h d", h=H),
                    AF.Copy)

            # ---- attention + projection ----
            proj_in = [[None, None], [None, None]]
            for ic in range(NIC):
                for g in range(2):
                    pv = [ps_acc.tile([64, 512], F32, tag="acc", name=f"pv{h}")
                          for h in range(4)]
                    for jc in range(NVC):
                        st = ps_st.tile([128, 2048], F32, tag="st", name="st")
                        for h in range(4):
                            nc.tensor.matmul(
                                st[:, 512 * h:512 * h + 512],
                                kT[g][32 * h:32 * h + 32, 128 * jc:128 * jc + 128],
                                qT[g][32 * h:32 * h + 32, 512 * ic:512 * ic + 512],
                                start=True, stop=True,
                                tile_position=(32 * h, 0))
                        p = work.tile([128, 2048], F32R, tag="p", name="p")
                        nc.scalar.activation(p[:], st[:], AF.Exp, scale=SCALE)
                        for h in range(4):
                            nc.tensor.matmul(
                                pv[h][:, :],
                                vb[:, jc, 4 * g + h, 0:64],
                                p[:, 512 * h:512 * h + 512],
                                start=(jc == 0), stop=(jc == NVC - 1))
                    pin = pers.tile([128, 512], F32R, name=f"pin{g}{ic}")
                    proj_in[g][ic] = pin
                    for h in range(4):
                        rinv = work.tile([32, 512], F32, tag="rinv", name="rinv")
                        nc.vector.reciprocal(rinv[:], pv[h][32:64, :])
                        nc.vector.tensor_mul(
                            pin[32 * h:32 * h + 32, :],
                            pv[h][0:32, :],
                            rinv[:])

            for ic in range(NIC):
                for m in range(2):
                    acc = ps_acc.tile([128, 512], F32, tag="acc", name="pracc")
                    for g in range(2):
                        nc.tensor.matmul(
                            acc[:], wproj[g][:, 128 * m:128 * m + 128],
                            proj_in[g][ic][:],
                            start=(g == 0), stop=False)
                    nc.tensor.matmul(acc[:], bproj[0:1, 128 * m:128 * m + 128],
                                     ones512[:], start=False, stop=True)
                    ot = work.tile([128, 512], F32, tag="ot", name="ot")
                    nc.vector.tensor_copy(ot[:], acc[:])
                    nc.sync.dma_start(
                        OUT[128 * m:128 * m + 128, 512 * ic:512 * ic + 512], ot[:])
            work_cm.__exit__(None, None, None)

    nc.compile()
    return nc


def _get_nc():
    if "nc" not in _CACHE:
        _CACHE["nc"] = _build_nc()
    return _CACHE["nc"]


def kernel(x, Wqkv, bqkv, Wproj, bproj):
    from concourse.bass_utils import run_bass_kernel_spmd

    x = np.ascontiguousarray(x, dtype=np.float32)
    in_maps = []
    for c in range(8):
        b, q = c // 4, c % 4
        in_maps.append({
            "xb": x[b],
            "xq": np.ascontiguousarray(x[b][:, q * NI:(q + 1) * NI]),
            "wqkv": np.ascontiguousarray(Wqkv, dtype=np.float32),
            "bqkv": np.ascontiguousarray(bqkv, dtype=np.float32).reshape(1, 3 * C),
            "wproj": np.ascontiguousarray(Wproj, dtype=np.float32),
            "bproj": np.ascontiguousarray(bproj, dtype=np.float32).reshape(1, C),
        })
    res = run_bass_kernel_spmd(_get_nc(), in_maps, list(range(8)))
    out = np.empty((B, C, N), dtype=np.float32)
    for c in range(8):
        b, q = c // 4, c % 4
        out[b][:, q * NI:(q + 1) * NI] = res.results[c]["out"]
    return out


# revision 11
# speedup vs baseline: 1.2914x; 1.2914x over previous
"""MultiHeadSelfAttention1D on 8 trn2 NeuronCores.

Sharding: core c handles batch b=c//4 and query-range [(c%4)*1024, +1024),
all 8 heads. Each core redundantly computes K/V for its batch (cheaper than
collectives) and produces a complete [C, 1024] slice of the output — no
cross-core communication.

Per-core dataflow (matmul operands bf16, fp32 PSUM accumulation):
  qT/kT via W.T@x (transposed projections, K=256 over 2 chunks)
  v in token-major layout via x.T@Wv, per head [v_h(32) | ones(32)] so the
    PV matmul emits out^T rows and a x32-replicated rowsum in one pass
  S^T[j,i] tiles via 4 row-packed K=32 matmuls (tile_position row groups)
  P = exp(SCALE*S^T) on ScalarE, one [128,2048] ACTIVATE per 4-head group,
    written directly as bf16
  out^T+rowsum via col-packed M=64 matmuls accumulating over j
  normalize on VectorE (reciprocal + mul), project with Wproj.
"""
import sys

sys.path.insert(0, '/opt/trn_rl_repo')

import numpy as np

B, C, N = 2, 256, 4096
H, HD = 8, 32
NI = N // 4          # queries per core
SCALE = HD ** -0.5

_CACHE = {}


def _build_nc():
    import concourse.tile as tile
    from concourse import bacc, mybir

    F32 = mybir.dt.float32
    BF16 = mybir.dt.bfloat16
    AF = mybir.ActivationFunctionType

    nc = bacc.Bacc("TRN2", target_bir_lowering=False, debug=False, num_devices=8)
    XB = nc.declare_dram_parameter("xb", [C, N], F32, isOutput=False)
    XQ = nc.declare_dram_parameter("xq", [C, NI], F32, isOutput=False)
    WQKV = nc.declare_dram_parameter("wqkv", [C, 3 * C], F32, isOutput=False)
    BQKV = nc.declare_dram_parameter("bqkv", [1, 3 * C], F32, isOutput=False)
    WPROJ = nc.declare_dram_parameter("wproj", [C, C], F32, isOutput=False)
    BPROJ = nc.declare_dram_parameter("bproj", [1, C], F32, isOutput=False)
    OUT = nc.declare_dram_parameter("out", [C, NI], F32, isOutput=True)

    NJC = N // 512       # 8 kT column chunks
    NVC = N // 128       # 32 v token chunks
    NIC = NI // 512      # 2 query chunks

    with tile.TileContext(nc) as tc:
        with (
            tc.tile_pool(name="persist", bufs=1) as pers,
            tc.tile_pool(name="work", bufs=2) as work,
            tc.tile_pool(name="ps_st", bufs=1, space="PSUM") as ps_st,
            tc.tile_pool(name="ps_acc", bufs=4, space="PSUM") as ps_acc,
        ):
            # ---- load inputs, casting fp32 -> bf16 via gpsimd DMA ----
            xb = [pers.tile([128, N], BF16, name=f"xb{i}") for i in range(2)]
            xq = [pers.tile([128, NI], BF16, name=f"xq{i}") for i in range(2)]
            wqkv = [pers.tile([128, 3 * C], BF16, name=f"wqkv{i}") for i in range(2)]
            wproj = [pers.tile([128, C], BF16, name=f"wproj{i}") for i in range(2)]
            bqkv = pers.tile([1, 3 * C], BF16)
            bproj = pers.tile([1, C], BF16)
            for i in range(2):
                nc.gpsimd.dma_start(xb[i][:], XB[128 * i:128 * i + 128, :])
                nc.gpsimd.dma_start(xq[i][:], XQ[128 * i:128 * i + 128, :])
                nc.gpsimd.dma_start(wqkv[i][:], WQKV[128 * i:128 * i + 128, :])
                nc.gpsimd.dma_start(wproj[i][:], WPROJ[128 * i:128 * i + 128, :])
            nc.gpsimd.dma_start(bqkv[:], BQKV[:])
            nc.gpsimd.dma_start(bproj[:], BPROJ[:])

            ones512 = pers.tile([1, 512], BF16)
            ones128 = pers.tile([1, 128], BF16)
            nc.vector.memset(ones512[:], 1.0)
            nc.vector.memset(ones128[:], 1.0)

            # ---- projections ----
            kT = [pers.tile([128, N], BF16, name=f"kT{g}") for g in range(2)]
            qT = [pers.tile([128, NI], BF16, name=f"qT{g}") for g in range(2)]
            # v: per token-chunk, per head: [v_h(32) | ones(32)] -> M=64 lhsT,
            # so the PV matmul emits out^T rows 0:32 and rowsum x32 rows 32:64.
            vb = pers.tile([128, NVC, H, 64], BF16)
            nc.vector.memset(vb[:, :, :, 32:64], 1.0)

            for g in range(2):
                co = C + 128 * g  # k block columns
                for jc in range(NJC):
                    acc = ps_acc.tile([128, 512], F32, tag="acc", name="kacc")
                    for cc in range(2):
                        nc.tensor.matmul(
                            acc[:], wqkv[cc][:, co:co + 128],
                            xb[cc][:, 512 * jc:512 * jc + 512],
                            start=(cc == 0), stop=False)
                    nc.tensor.matmul(acc[:], bqkv[0:1, co:co + 128], ones512[:],
                                     start=False, stop=True)
                    nc.vector.tensor_copy(kT[g][:, 512 * jc:512 * jc + 512], acc[:])
                for ic in range(NIC):
                    acc = ps_acc.tile([128, 512], F32, tag="acc", name="qacc")
                    for cc in range(2):
                        nc.tensor.matmul(
                            acc[:], wqkv[cc][:, 128 * g:128 * g + 128],
                            xq[cc][:, 512 * ic:512 * ic + 512],
                            start=(cc == 0), stop=False)
                    nc.tensor.matmul(acc[:], bqkv[0:1, 128 * g:128 * g + 128], ones512[:],
                                     start=False, stop=True)
                    nc.vector.tensor_copy(qT[g][:, 512 * ic:512 * ic + 512], acc[:])

            for jc in range(NVC):
                acc = ps_acc.tile([128, C], F32, tag="acc", name="vacc")
                for cc in range(2):
                    nc.tensor.matmul(
                        acc[:], xb[cc][:, 128 * jc:128 * jc + 128],
                        wqkv[cc][:, 2 * C:3 * C],
                        start=(cc == 0), stop=False)
                nc.tensor.matmul(acc[:], ones128[:], bqkv[0:1, 2 * C:3 * C],
                                 start=False, stop=True)
                nc.vector.tensor_copy(
                    vb[:, jc, :, 0:32],
                    acc[:].rearrange("p (h d) -> p h d", h=H))

            # ---- attention + projection ----
            proj_in = [[None, None], [None, None]]
            for ic in range(NIC):
                for g in range(2):
                    # 2 banks, each holding 2 col-packed M=64 head outputs
                    pv = [ps_acc.tile([128, 512], F32, tag="acc", name=f"pv{k}")
                          for k in range(2)]
                    for jc in range(NVC):
                        st = ps_st.tile([128, 2048], F32, tag="st", name="st")
                        for h in range(4):
                            nc.tensor.matmul(
                                st[:, 512 * h:512 * h + 512],
                                kT[g][32 * h:32 * h + 32, 128 * jc:128 * jc + 128],
                                qT[g][32 * h:32 * h + 32, 512 * ic:512 * ic + 512],
                                start=True, stop=True,
                                tile_position=(32 * h, 0))
                        p = work.tile([128, 2048], BF16, tag="p", name="p")
                        nc.scalar.activation(p[:], st[:], AF.Exp, scale=SCALE)
                        for k in range(2):
                            for a in range(2):
                                h = 2 * k + a
                                nc.tensor.matmul(
                                    pv[k][64 * a:64 * a + 64, :],
                                    vb[:, jc, 4 * g + h, :],
                                    p[:, 512 * h:512 * h + 512],
                                    start=(jc == 0), stop=(jc == NVC - 1),
                                    tile_position=(0, 64 * a),
                                    skip_group_check=True)
                    pin = pers.tile([128, 512], BF16, name=f"pin{g}{ic}")
                    proj_in[g][ic] = pin
                    for k in range(2):
                        for a in range(2):
                            h = 2 * k + a
                            rinv = work.tile([32, 512], F32, tag="rinv", name="rinv")
                            nc.vector.reciprocal(rinv[:], pv[k][64 * a + 32:64 * a + 64, :])
                            nc.vector.tensor_mul(
                                pin[32 * h:32 * h + 32, :],
                                pv[k][64 * a:64 * a + 32, :],
                                rinv[:])

            for ic in range(NIC):
                for m in range(2):
                    acc = ps_acc.tile([128, 512], F32, tag="acc", name="pracc")
                    for g in range(2):
                        nc.tensor.matmul(
                            acc[:], wproj[g][:, 128 * m:128 * m + 128],
                            proj_in[g][ic][:],
                            start=(g == 0), stop=False)
                    nc.tensor.matmul(acc[:], bproj[0:1, 128 * m:128 * m + 128],
                                     ones512[:], start=False, stop=True)
                    ot = work.tile([128, 512], F32, tag="ot", name="ot")
                    nc.vector.tensor_copy(ot[:], acc[:])
                    nc.sync.dma_start(
                        OUT[128 * m:128 * m + 128, 512 * ic:512 * ic + 512], ot[:])

    nc.compile()
    return nc


def _get_nc():
    if "nc" not in _CACHE:
        _CACHE["nc"] = _build_nc()
    return _CACHE["nc"]


def kernel(x, Wqkv, bqkv, Wproj, bproj):
    from concourse.bass_utils import run_bass_kernel_spmd

    x = np.ascontiguousarray(x, dtype=np.float32)
    in_maps = []
    for c in range(8):
        b, q = c // 4, c % 4
        in_maps.append({
            "xb": x[b],
            "xq": np.ascontiguousarray(x[b][:, q * NI:(q + 1) * NI]),
            "wqkv": np.ascontiguousarray(Wqkv, dtype=np.float32),
            "bqkv": np.ascontiguousarray(bqkv, dtype=np.float32).reshape(1, 3 * C),
            "wproj": np.ascontiguousarray(Wproj, dtype=np.float32),
            "bproj": np.ascontiguousarray(bproj, dtype=np.float32).reshape(1, C),
        })
    res = run_bass_kernel_spmd(_get_nc(), in_maps, list(range(8)))
    out = np.empty((B, C, N), dtype=np.float32)
    for c in range(8):
        b, q = c // 4, c % 4
        out[b][:, q * NI:(q + 1) * NI] = res.results[c]["out"]
    return out


# revision 13
# speedup vs baseline: 1.4681x; 1.1368x over previous
"""MultiHeadSelfAttention1D on 8 trn2 NeuronCores.

Sharding: core c handles batch b=c//4 and query-range [(c%4)*1024, +1024),
all 8 heads. Each core redundantly computes K/V for its batch (cheaper than
collectives) and produces a complete [C, 1024] slice of the output — no
cross-core communication.

Per-core dataflow (matmul operands bf16, fp32 PSUM accumulation):
  qT/kT via W.T@x (transposed projections), v token-major via x.T@Wv with a
  per-head [v_h(32) | ones(32)] block so one M=64 matmul emits both out^T
  and a x32-replicated rowsum. S^T[j,i] via 4 row-packed K=32 matmuls
  (tile_position row groups); P = exp(SCALE*S^T) as one [128,2048] bf16
  ACTIVATE; PV col-packed in M=64 pairs accumulating over j; normalize on
  VectorE; biases folded into per-partition tensor_scalar adds (no PE cost).
  Emission pipelines exp_j with S^T_{j+1} and PV_j so ScalarE stays busy
  and TensorE sustains activity (HAM stays at 2.4 GHz).
"""
import sys

sys.path.insert(0, '/opt/trn_rl_repo')

import numpy as np

B, C, N = 2, 256, 4096
H, HD = 8, 32
NI = N // 4          # queries per core
SCALE = HD ** -0.5

_CACHE = {}


def _build_nc():
    import concourse.tile as tile
    from concourse import bacc, mybir

    F32 = mybir.dt.float32
    BF16 = mybir.dt.bfloat16
    AF = mybir.ActivationFunctionType

    nc = bacc.Bacc("TRN2", target_bir_lowering=False, debug=False, num_devices=8)
    XB = nc.declare_dram_parameter("xb", [C, N], F32, isOutput=False)
    XQ = nc.declare_dram_parameter("xq", [C, NI], F32, isOutput=False)
    WQKV = nc.declare_dram_parameter("wqkv", [C, 3 * C], F32, isOutput=False)
    BQKVT = nc.declare_dram_parameter("bqkvT", [128, 6], F32, isOutput=False)
    WPROJ = nc.declare_dram_parameter("wproj", [C, C], F32, isOutput=False)
    BPROJT = nc.declare_dram_parameter("bprojT", [128, 2], F32, isOutput=False)
    OUT = nc.declare_dram_parameter("out", [C, NI], F32, isOutput=True)

    NVC = N // 128       # 32 v token chunks / S^T j chunks
    NIC = NI // 512      # 2 query chunks

    with tile.TileContext(nc) as tc:
        with (
            tc.tile_pool(name="persist", bufs=1) as pers,
            tc.tile_pool(name="work", bufs=2) as work,
            tc.tile_pool(name="ps_st", bufs=1, space="PSUM") as ps_st,
            tc.tile_pool(name="ps_acc", bufs=4, space="PSUM") as ps_acc,
        ):
            # ---- load inputs, casting fp32 -> bf16 via gpsimd DMA ----
            wqkv = [pers.tile([128, 3 * C], BF16, name=f"wqkv{i}") for i in range(2)]
            xq = [pers.tile([128, NI], BF16, name=f"xq{i}") for i in range(2)]
            xb = [pers.tile([128, N], BF16, name=f"xb{i}") for i in range(2)]
            wproj = [pers.tile([128, C], BF16, name=f"wproj{i}") for i in range(2)]
            bqkvT = pers.tile([128, 6], F32)
            bprojT = pers.tile([128, 2], F32)
            for i in range(2):
                nc.gpsimd.dma_start(wqkv[i][:], WQKV[128 * i:128 * i + 128, :])
                nc.gpsimd.dma_start(xq[i][:], XQ[128 * i:128 * i + 128, :])
                nc.gpsimd.dma_start(xb[i][:], XB[128 * i:128 * i + 128, :])
            nc.sync.dma_start(bqkvT[:], BQKVT[:])
            nc.sync.dma_start(bprojT[:], BPROJT[:])
            for i in range(2):
                nc.gpsimd.dma_start(wproj[i][:], WPROJ[128 * i:128 * i + 128, :])

            # persistent operand tiles
            kTt = [[pers.tile([128, 1024], BF16, name=f"kT{g}_{t}") for t in range(4)]
                   for g in range(2)]
            qT = [pers.tile([128, NI], BF16, name=f"qT{g}") for g in range(2)]
            vb = [pers.tile([128, H, 64], BF16, name=f"vb{jc}") for jc in range(NVC)]
            pin = [[pers.tile([128, 512], BF16, name=f"pin{g}{ic}") for ic in range(NIC)]
                   for g in range(2)]

            def emit_qT(g):
                for ic in range(NIC):
                    acc = ps_acc.tile([128, 512], F32, tag="acc", name="qacc")
                    for cc in range(2):
                        nc.tensor.matmul(
                            acc[:], wqkv[cc][:, 128 * g:128 * g + 128],
                            xq[cc][:, 512 * ic:512 * ic + 512],
                            start=(cc == 0), stop=(cc == 1))
                    nc.vector.tensor_scalar_add(
                        qT[g][:, 512 * ic:512 * ic + 512], acc[:], bqkvT[:, g:g + 1])

            def emit_kT(g, t, nn):
                co = C + 128 * g
                acc = ps_acc.tile([128, 512], F32, tag="acc", name="kacc")
                for cc in range(2):
                    nc.tensor.matmul(
                        acc[:], wqkv[cc][:, co:co + 128],
                        xb[cc][:, 1024 * t + 512 * nn:1024 * t + 512 * nn + 512],
                        start=(cc == 0), stop=(cc == 1))
                nc.vector.tensor_scalar_add(
                    kTt[g][t][:, 512 * nn:512 * nn + 512], acc[:], bqkvT[:, 2 + g:3 + g])

            def emit_vb(jc):
                nc.vector.memset(vb[jc][:, :, 32:64], 1.0)
                acc = ps_acc.tile([128, C], F32, tag="acc", name="vacc")
                for cc in range(2):
                    nc.tensor.matmul(
                        acc[:], xb[cc][:, 128 * jc:128 * jc + 128],
                        wqkv[cc][:, 2 * C:3 * C],
                        start=(cc == 0), stop=(cc == 1))
                nc.vector.tensor_copy(
                    vb[jc][:, :, 0:32],
                    acc[:].rearrange("p (h d) -> p h d", h=H))

            emit_qT(0)
            for t in range(4):
                for nn in range(2):
                    emit_kT(0, t, nn)
            emit_vb(0)
            emit_vb(1)

            # deferred-production schedule inside attention combos:
            # combo 0 (g0,ic0): vb chunks 2..31 ; combo 1 (g0,ic1): kT[1], qT[1]
            combos = [(0, 0), (0, 1), (1, 0), (1, 1)]

            def interleave(t_idx, j):
                if t_idx == 0 and j + 2 < NVC:
                    emit_vb(j + 2)
                if t_idx == 1:
                    if j % 4 == 0:
                        tt = j // 8
                        emit_kT(1, tt, (j // 4) % 2)
                    elif j == 29:
                        emit_qT(1)

            for t_idx, (g, ic) in enumerate(combos):
                pv = [ps_acc.tile([128, 512], F32, tag="acc", name=f"pv{k}")
                      for k in range(2)]

                def st_mms(j):
                    st = ps_st.tile([128, 2048], F32, tag="st", name="st")
                    for h in range(4):
                        nc.tensor.matmul(
                            st[:, 512 * h:512 * h + 512],
                            kTt[g][j // 8][32 * h:32 * h + 32,
                                           128 * (j % 8):128 * (j % 8) + 128],
                            qT[g][32 * h:32 * h + 32, 512 * ic:512 * ic + 512],
                            start=True, stop=True,
                            tile_position=(32 * h, 0))
                    return st

                st = st_mms(0)
                for j in range(NVC):
                    p = work.tile([128, 2048], BF16, tag="p", name="p")
                    nc.scalar.activation(p[:], st[:], AF.Exp, scale=SCALE)
                    interleave(t_idx, j)
                    if j + 1 < NVC:
                        st = st_mms(j + 1)
                    for k in range(2):
                        for a in range(2):
                            h = 2 * k + a
                            nc.tensor.matmul(
                                pv[k][64 * a:64 * a + 64, :],
                                vb[j][:, 4 * g + h, :],
                                p[:, 512 * h:512 * h + 512],
                                start=(j == 0), stop=(j == NVC - 1),
                                tile_position=(0, 64 * a),
                                skip_group_check=True)

                for k in range(2):
                    for a in range(2):
                        h = 2 * k + a
                        rinv = work.tile([32, 512], F32, tag="rinv", name="rinv")
                        nc.vector.reciprocal(rinv[:], pv[k][64 * a + 32:64 * a + 64, :])
                        nc.vector.tensor_mul(
                            pin[g][ic][32 * h:32 * h + 32, :],
                            pv[k][64 * a:64 * a + 32, :],
                            rinv[:])
                nc.vector.tensor_scalar_add(
                    pin[g][ic][:], pin[g][ic][:], bqkvT[:, 4 + g:5 + g])

                if t_idx in (2, 3):
                    pic = t_idx - 2  # projection for ic after both g done
                    for m in range(2):
                        acc = ps_acc.tile([128, 512], F32, tag="acc", name="pracc")
                        for gg in range(2):
                            nc.tensor.matmul(
                                acc[:], wproj[gg][:, 128 * m:128 * m + 128],
                                pin[gg][pic][:],
                                start=(gg == 0), stop=(gg == 1))
                        ot = work.tile([128, 512], F32, tag="ot", name="ot")
                        nc.vector.tensor_scalar_add(ot[:], acc[:], bprojT[:, m:m + 1])
                        nc.sync.dma_start(
                            OUT[128 * m:128 * m + 128, 512 * pic:512 * pic + 512], ot[:])

    nc.compile()
    return nc


def _get_nc():
    if "nc" not in _CACHE:
        _CACHE["nc"] = _build_nc()
    return _CACHE["nc"]


def _prep_in_maps(x, Wqkv, bqkv, Wproj, bproj):
    x = np.ascontiguousarray(x, dtype=np.float32)
    bqkvT = np.ascontiguousarray(
        np.asarray(bqkv, dtype=np.float32).reshape(6, 128).T)
    bprojT = np.ascontiguousarray(
        np.asarray(bproj, dtype=np.float32).reshape(2, 128).T)
    wqkv = np.ascontiguousarray(Wqkv, dtype=np.float32)
    wproj = np.ascontiguousarray(Wproj, dtype=np.float32)
    in_maps = []
    for c in range(8):
        b, q = c // 4, c % 4
        in_maps.append({
            "xb": x[b],
            "xq": np.ascontiguousarray(x[b][:, q * NI:(q + 1) * NI]),
            "wqkv": wqkv,
            "bqkvT": bqkvT,
            "wproj": wproj,
            "bprojT": bprojT,
        })
    return in_maps


def kernel(x, Wqkv, bqkv, Wproj, bproj):
    from concourse.bass_utils import run_bass_kernel_spmd

    in_maps = _prep_in_maps(x, Wqkv, bqkv, Wproj, bproj)
    res = run_bass_kernel_spmd(_get_nc(), in_maps, list(range(8)))
    out = np.empty((B, C, N), dtype=np.float32)
    for c in range(8):
        b, q = c // 4, c % 4
        out[b][:, q * NI:(q + 1) * NI] = res.results[c]["out"]
    return out
